# revision 1
# baseline (speedup 1.0000x reference)
"""Trainium2 Bass kernel for nn_CoordinateDecoder.

Computation (see reference): posenc(coords) ++ trilinear-pyramid-sampled
features -> 5-layer MLP (gelu-tanh approx, skip concat at depth 2, tanh out).

Strategy:
  - Data-parallel over B: core b handles batch image b (coords/weights shared).
  - Bilinear pyramid sampling is done ON THE TENSOR ENGINE: samples are
    host-sorted by their continuous y coordinate, so for every pyramid level
    the samples that read a given 2-row band of the grid are contiguous.
    Sampling then becomes, per y-bucket, a matmul
        out[256ch, n_run] = RP[bucket][128 grid-cells, 256ch]^T @ S[128, n_run]
    where S holds the 4 bilinear weights per sample (built dense on host,
    shipped bf16).  This produces features directly in feature-major layout
    (channels on partitions), which is what the MLP matmuls need.
  - MLP runs in bf16 (fp32 PSUM accumulation), weights stationary, N=512
    moving tiles.  Gelu (tanh approx) + bias fused on the scalar engine.
  - Host does only O(N) / O(grid) prep: pyramid resize (134 MMAC), posenc,
    bilinear index/weight computation, argsort, packing.  All heavy compute
    (80 GMAC of matmul) is on device.
"""

import numpy as np
import ml_dtypes

BF16 = ml_dtypes.bfloat16

B, H, W, C = 8, 64, 64, 256
N = 16384
NUM_FREQS = 10
MLP_WIDTH = 256
IN_DIM = 2 + 4 * NUM_FREQS + 3 * C  # 810

NSUP = 8            # column supers
SUP = N // NSUP     # 2048
NCH = 4             # 512-chunks per super
CH = 512

LEVEL_SIZES = [64, 32, 16]
# per-level k-layout of the RP (row-pair) stationary tensors:
#   L0: bucket g in [0,63): partitions r*64+x  = grid rows (g, g+1)
#   L1: bucket b in [0,11): partitions r*32+x  = grid rows (3b .. 3b+3)
#   L2: quad   q in [0,4):  partitions 32*rb + dy*16 + x = rows (4q+rb, 4q+rb+1)
N_BUCKETS = [63, 11, 4]


def _resize_matrix(out_size: int, in_size: int) -> np.ndarray:
    """Row-resize operator of jax.image.resize(..., 'bilinear') (antialias).
    Returns M [out_size, in_size] with resized = M @ x."""
    scale = out_size / in_size
    inv_scale = 1.0 / scale
    kernel_scale = max(inv_scale, 1.0)
    sample_f = (np.arange(out_size, dtype=np.float64) + 0.5) * inv_scale - 0.5
    x = np.abs(sample_f[None, :] - np.arange(in_size, dtype=np.float64)[:, None])
    x = x / kernel_scale
    w = np.where(x < 1.0, 1.0 - x, 0.0)
    total = w.sum(axis=0, keepdims=True)
    w = np.where(
        np.abs(total) > 1000.0 * np.finfo(np.float32).eps,
        w / np.where(total != 0.0, total, 1.0),
        0.0,
    )
    w = np.where(
        ((sample_f >= -0.5) & (sample_f <= in_size - 0.5))[None, :], w, 0.0
    )
    return w.T.astype(np.float32)  # [out, in]


def _posenc_t(coords: np.ndarray) -> np.ndarray:
    """Transposed positional encoding [42, n] fp32, matching reference order."""
    freqs = (2.0 ** np.arange(NUM_FREQS, dtype=np.float32)) * np.float32(np.pi)
    parts = [coords.T.astype(np.float32)]
    for f in freqs:
        parts.append(np.sin(coords.T * f).astype(np.float32))
        parts.append(np.cos(coords.T * f).astype(np.float32))
    return np.concatenate(parts, axis=0)  # [42, n]


def _bilinear(c01: np.ndarray, size: int):
    """c01 [n] in [0,1] -> (i0, frac) fp32 like the reference's fp32 math."""
    cr = (c01 * np.float32(size - 1)).astype(np.float32)
    i0 = np.floor(cr).astype(np.int64)
    i0 = np.clip(i0, 0, size - 2)
    f = cr - i0.astype(np.float32)
    return i0, f.astype(np.float32)


def _host_prep(feature_grid, coords, w0, b0, w1, b1, w2, b2, w3, b3, w_out, b_out):
    """All host-side packing. Returns (shared_map, per_core_maps, perm, runs)."""
    fg = np.asarray(feature_grid, dtype=np.float32)
    coords = np.asarray(coords, dtype=np.float32)

    # ---- sort samples by continuous y so every level's y-buckets are runs ----
    c01 = (coords + np.float32(1.0)) / np.float32(2.0)  # [N,2] (y, x)
    perm = np.argsort(c01[:, 0], kind="stable")
    c01s = c01[perm]
    coords_s = coords[perm]

    # ---- per-level bilinear indices / weights / buckets -----------------------
    y0, fy, x0, fx, buckets = [], [], [], [], []
    for li, S in enumerate(LEVEL_SIZES):
        yi, fyi = _bilinear(c01s[:, 0], S)
        xi, fxi = _bilinear(c01s[:, 1], S)
        y0.append(yi); fy.append(fyi); x0.append(xi); fx.append(fxi)
        if li == 0:
            buckets.append(yi.copy())
        elif li == 1:
            buckets.append(yi // 3)
        else:
            buckets.append(yi // 4)

    # ---- dense S^T matrices [128, N] bf16 ------------------------------------
    s_t = []
    for li in range(3):
        Sm = np.zeros((N, 128), np.float32)
        wtl = (1 - fy[li]) * (1 - fx[li])
        wtr = (1 - fy[li]) * fx[li]
        wbl = fy[li] * (1 - fx[li])
        wbr = fy[li] * fx[li]
        j = np.arange(N)
        if li == 0:
            ktop = x0[li]
            kbot = 64 + x0[li]
        elif li == 1:
            dy_loc = y0[li] - 3 * buckets[li]
            ktop = dy_loc * 32 + x0[li]
            kbot = (dy_loc + 1) * 32 + x0[li]
        else:
            rb = y0[li] - 4 * buckets[li]
            ktop = rb * 32 + x0[li]
            kbot = rb * 32 + 16 + x0[li]
        Sm[j, ktop] = wtl
        Sm[j, ktop + 1] = wtr
        Sm[j, kbot] = wbl
        Sm[j, kbot + 1] = wbr
        s_t.append(np.ascontiguousarray(Sm.T).astype(BF16))

    # ---- bucket runs, split at CH boundaries ---------------------------------
    runs = []  # runs[level][chunk] = list of (bucket, off_in_chunk, length)
    for li in range(3):
        bk = buckets[li]
        per_chunk = [[] for _ in range(N // CH)]
        start = 0
        while start < N:
            g = bk[start]
            end = start
            while end < N and bk[end] == g:
                end += 1
            # split [start, end) at chunk boundaries
            p = start
            while p < end:
                ci = p // CH
                q = min(end, (ci + 1) * CH)
                per_chunk[ci].append((int(g), p - ci * CH, q - p))
                p = q
            start = end
        runs.append(per_chunk)

    # ---- pyramid + row-pair (RP) tensors per core ----------------------------
    R1 = _resize_matrix(32, 64)
    R2 = _resize_matrix(16, 64)
    g1 = np.einsum("ph,qw,bhwc->bpqc", R1, R1, fg, optimize=True)
    g2 = np.einsum("ph,qw,bhwc->bpqc", R2, R2, fg, optimize=True)

    def rp_tensors(g0b, g1b, g2b):
        # L0: [128, 63*256]: bucket g -> rows (g, g+1), partitions r*64+x
        rp0 = np.zeros((128, 63 * 256), np.float32)
        for g in range(63):
            blk = g0b[g:g + 2]                      # [2, 64, 256]
            rp0[:, g * 256:(g + 1) * 256] = blk.reshape(128, 256)
        # L1: [128, 11*256]: bucket b -> rows 3b..3b+3 (pad past row 31)
        rp1 = np.zeros((128, 11 * 256), np.float32)
        for b in range(11):
            rows = g1b[3 * b:3 * b + 4]             # up to [4, 32, 256]
            blk = np.zeros((4, 32, 256), np.float32)
            blk[:rows.shape[0]] = rows
            rp1[:, b * 256:(b + 1) * 256] = blk.reshape(128, 256)
        # L2: [128, 4*256]: quad q, block rb -> rows (4q+rb, 4q+rb+1)
        rp2 = np.zeros((128, 4 * 256), np.float32)
        for q in range(4):
            blk = np.zeros((4, 2, 16, 256), np.float32)
            for rb in range(4):
                rows = g2b[4 * q + rb:4 * q + rb + 2]
                blk[rb, :rows.shape[0]] = rows
            rp2[:, q * 256:(q + 1) * 256] = blk.reshape(128, 256)
        return rp0.astype(BF16), rp1.astype(BF16), rp2.astype(BF16)

    per_core = []
    for b in range(B):
        rp0, rp1, rp2 = rp_tensors(fg[b], g1[b], g2[b])
        per_core.append({"rp0": rp0, "rp1": rp1, "rp2": rp2})

    # ---- posenc (padded to a full 128-row k-tile) ----------------------------
    enc = np.zeros((128, N), np.float32)
    enc[:42] = _posenc_t(coords_s)
    enc = enc.astype(BF16)

    # ---- weights: reorder rows into the device k-layout, pack [128, kt*M] ----
    w0 = np.asarray(w0, np.float32); w1 = np.asarray(w1, np.float32)
    w2 = np.asarray(w2, np.float32); w3 = np.asarray(w3, np.float32)
    w_out = np.asarray(w_out, np.float32)

    def pack(wd):  # [Ktot, M] -> [128, (Ktot/128) * M], k-tile major
        K, M = wd.shape
        assert K % 128 == 0
        return np.ascontiguousarray(
            wd.reshape(K // 128, 128, M).transpose(1, 0, 2).reshape(128, -1)
        )

    w0d = np.zeros((896, 256), np.float32)
    w0d[0:42] = w0[0:42]          # enc
    w0d[128:384] = w0[42:298]     # L0
    w0d[384:640] = w0[298:554]    # L1
    w0d[640:896] = w0[554:810]    # L2
    w3d = np.zeros((1152, 256), np.float32)
    w3d[0:256] = w3[0:256]        # h
    w3d[256:298] = w3[256:298]    # enc
    w3d[384:640] = w3[298:554]    # L0
    w3d[640:896] = w3[554:810]    # L1
    w3d[896:1152] = w3[810:1066]  # L2
    woutd = np.zeros((256, 3), np.float32)
    woutd[:] = w_out

    shared = {
        "s0t": s_t[0], "s1t": s_t[1], "s2t": s_t[2], "enc": enc,
        "w0": pack(w0d).astype(BF16), "w1": pack(w1).astype(BF16),
        "w2": pack(w2).astype(BF16), "w3": pack(w3d).astype(BF16),
        "wout": pack(woutd).astype(BF16),
        "b0": np.asarray(b0, np.float32).reshape(2, 128).T.copy(),
        "b1": np.asarray(b1, np.float32).reshape(2, 128).T.copy(),
        "b2": np.asarray(b2, np.float32).reshape(2, 128).T.copy(),
        "b3": np.asarray(b3, np.float32).reshape(2, 128).T.copy(),
        "bout": np.asarray(b_out, np.float32).reshape(3, 1).copy(),
    }
    return shared, per_core, perm, runs


_DRAM_SPECS = [
    # name, shape, np dtype
    ("rp0", (128, 63 * 256), BF16),
    ("rp1", (128, 11 * 256), BF16),
    ("rp2", (128, 4 * 256), BF16),
    ("s0t", (128, N), BF16),
    ("s1t", (128, N), BF16),
    ("s2t", (128, N), BF16),
    ("enc", (128, N), BF16),
    ("w0", (128, 7 * 256), BF16),
    ("w1", (128, 2 * 256), BF16),
    ("w2", (128, 2 * 256), BF16),
    ("w3", (128, 9 * 256), BF16),
    ("wout", (128, 2 * 3), BF16),
    ("b0", (128, 2), np.float32),
    ("b1", (128, 2), np.float32),
    ("b2", (128, 2), np.float32),
    ("b3", (128, 2), np.float32),
    ("bout", (3, 1), np.float32),
]


def _build_nc(runs):
    """Build the Bacc program (shared by all cores; per-core data differs)."""
    from contextlib import ExitStack

    import concourse.bacc as bacc
    import concourse.mybir as mybir
    import concourse.tile as tile

    bf16 = mybir.dt.bfloat16
    f32 = mybir.dt.float32
    GELU = mybir.ActivationFunctionType.Gelu_apprx_tanh
    TANH = mybir.ActivationFunctionType.Tanh

    nc = bacc.Bacc("TRN2", debug=False, target_bir_lowering=False)

    dram = {}
    for name, shape, npdt in _DRAM_SPECS:
        dram[name] = nc.dram_tensor(
            name, list(shape), mybir.dt.from_np(np.dtype(npdt)), kind="ExternalInput"
        )
    out_dram = nc.dram_tensor("out_t", [3, N], f32, kind="ExternalOutput")

    with tile.TileContext(nc) as tc, ExitStack() as ctx:
        const = ctx.enter_context(tc.tile_pool(name="const", bufs=1))
        spool = ctx.enter_context(tc.tile_pool(name="stream", bufs=2))
        xtpool = ctx.enter_context(tc.tile_pool(name="xt", bufs=2))
        hpool = ctx.enter_context(tc.tile_pool(name="h", bufs=5))
        opool = ctx.enter_context(tc.tile_pool(name="osb", bufs=2))
        ps_samp = ctx.enter_context(tc.tile_pool(name="ps_samp", bufs=3, space="PSUM"))
        ps_mlp = ctx.enter_context(tc.tile_pool(name="ps_mlp", bufs=4, space="PSUM"))
        ps_out = ctx.enter_context(tc.tile_pool(name="ps_out", bufs=1, space="PSUM"))

        # ---- static tensors ---------------------------------------------------
        st = {}
        # load order matters: small rp tensors first so sampling (L2, L1)
        # can start while the 4MB rp0 is still in flight; rp0 is split into
        # 4 independent quarter-loads so low buckets unblock early.
        order = ["rp2", "rp1", "rp0",
                 "w0", "w1", "w2", "w3", "wout", "b0", "b1", "b2", "b3", "bout"]
        specs = {n: (s, d) for n, s, d in _DRAM_SPECS}
        for name in order:
            if name not in specs:
                continue
            shape, npdt = specs[name]
            t = const.tile(list(shape), mybir.dt.from_np(np.dtype(npdt)), tag=name)
            if name == "rp0":
                q = shape[1] // 4
                for i in range(4):
                    nc.sync.dma_start(t[:, i * q:(i + 1) * q],
                                      dram[name][:, i * q:(i + 1) * q])
            else:
                nc.sync.dma_start(t[:, :], dram[name][:, :])
            st[name] = t

        rp = [st["rp0"], st["rp1"], st["rp2"]]
        wmlp = [st["w0"], st["w1"], st["w2"], st["w3"]]
        bmlp = [st["b0"], st["b1"], st["b2"], st["b3"]]
        KT = [7, 2, 2, 9]

        for s in range(NSUP):
            lo = s * SUP
            sl = slice(lo, lo + SUP)
            s_tiles = []
            for nm in ("s0t", "s1t", "s2t"):
                t = spool.tile([128, SUP], bf16, tag=nm)
                nc.sync.dma_start(t[:, :], dram[nm][:, sl])
                s_tiles.append(t)

            # X^T for this super: k-tiles [enc, L0a, L0b, L1a, L1b, L2a, L2b]
            xt = xtpool.tile([128, 7 * SUP], bf16, tag="xt")
            nc.sync.dma_start(xt[:, 0:SUP], dram["enc"][:, sl])

            # ---- sampling: per (m-tile, level, chunk) -------------------------
            for m in range(2):
                for li in range(3):
                    for ch in range(NCH):
                        p = ps_samp.tile([128, CH], f32, tag="ps_samp")
                        for (g, off, ln) in runs[li][s * NCH + ch]:
                            nc.tensor.matmul(
                                p[:, off:off + ln],
                                rp[li][:, g * 256 + m * 128: g * 256 + m * 128 + 128],
                                s_tiles[li][:, ch * CH + off: ch * CH + off + ln],
                                start=True, stop=True,
                            )
                        dst = (1 + 2 * li + m) * SUP + ch * CH
                        nc.vector.tensor_copy(xt[:, dst:dst + CH], p[:, :])

            # ---- MLP ---------------------------------------------------------
            def dense(layer, rhs_fn):
                h = hpool.tile([128, 2 * SUP], bf16, tag="h")
                for m in range(2):
                    pss = [ps_mlp.tile([128, CH], f32, tag="ps_mlp", name=f"ps_mlp_{layer}_{m}_{i}")
                           for i in range(NCH)]
                    for kt in range(KT[layer]):
                        lhsT = wmlp[layer][:, kt * 256 + m * 128:
                                           kt * 256 + m * 128 + 128]
                        for ns in range(NCH):
                            nc.tensor.matmul(
                                pss[ns][:, :], lhsT, rhs_fn(kt, ns),
                                start=(kt == 0), stop=(kt == KT[layer] - 1),
                            )
                    for ns in range(NCH):
                        nc.scalar.activation(
                            h[:, m * SUP + ns * CH: m * SUP + ns * CH + CH],
                            pss[ns][:, :], GELU, bias=bmlp[layer][:, m:m + 1],
                        )
                return h

            h0 = dense(0, lambda kt, ns: xt[:, kt * SUP + ns * CH: kt * SUP + ns * CH + CH])
            h1 = dense(1, lambda kt, ns: h0[:, kt * SUP + ns * CH: kt * SUP + ns * CH + CH])
            h2 = dense(2, lambda kt, ns: h1[:, kt * SUP + ns * CH: kt * SUP + ns * CH + CH])

            def rhs3(kt, ns):
                src = h2 if kt < 2 else xt
                k = kt if kt < 2 else kt - 2
                return src[:, k * SUP + ns * CH: k * SUP + ns * CH + CH]

            h3 = dense(3, rhs3)

            # ---- output layer -------------------------------------------------
            osb = opool.tile([3, SUP], f32, tag="osb")
            for ns in range(NCH):
                po = ps_out.tile([128, CH], f32, tag="ps_out")
                for kt in range(2):
                    nc.tensor.matmul(
                        po[:3, :],
                        st["wout"][:, kt * 3:(kt + 1) * 3],
                        h3[:, kt * SUP + ns * CH: kt * SUP + ns * CH + CH],
                        start=(kt == 0), stop=(kt == 1),
                    )
                nc.scalar.activation(
                    osb[:, ns * CH:(ns + 1) * CH], po[:3, :], TANH,
                    bias=st["bout"][:, 0:1],
                )
            nc.sync.dma_start(out_dram[:, sl], osb[:, :])

    nc.compile()
    return nc


def kernel(feature_grid, coords, w0, b0, w1, b1, w2, b2, w3, b3, w_out, b_out,
           _run_opts=None):
    from concourse.bass_utils import run_bass_kernel_spmd

    shared, per_core, perm, runs = _host_prep(
        feature_grid, coords, w0, b0, w1, b1, w2, b2, w3, b3, w_out, b_out)

    nc = _build_nc(runs)

    in_maps = []
    for b in range(B):
        m = dict(shared)
        m.update(per_core[b])
        in_maps.append(m)

    res = run_bass_kernel_spmd(
        nc, in_maps, core_ids=list(range(B)), **(_run_opts or {})
    )

    out = np.empty((B, N, 3), np.float32)
    inv = perm  # out_sorted column j corresponds to original sample perm[j]
    for b in range(B):
        out[b, inv, :] = res.results[b]["out_t"].T
    if _run_opts is not None:
        kernel._last_result = res  # for test harness introspection
    return out



# revision 4
# speedup vs baseline: 1.2886x; 1.2886x over previous
"""Trainium2 Bass kernel for nn_CoordinateDecoder (v2: projected-grid sampling).

Computation (see reference): posenc(coords) ++ bilinear-pyramid-sampled
features -> 5-layer MLP (gelu tanh-approx, skip concat at depth 2, tanh out).

Key idea v2: bilinear sampling is LINEAR in the grid, so
    w0_feat^T . bilerp(G_l, p) = bilerp(G_l @ w0_feat, p).
The host projects each pyramid level's grid through the layer-0 and layer-3
feature weight blocks; the device's sampling matmuls then directly produce the
layer-0 / layer-3 pre-activations in PSUM. This removes the two wide (k=810)
MLP matmuls entirely - they are replaced by a second sampling pass.
The posenc contribution becomes a host-computed per-sample bias [256, N]
added on the (otherwise idle) vector engine before each gelu.

  - Data-parallel over B: core b handles batch image b (coords/weights shared).
  - Samples host-sorted by continuous y; per pyramid level, samples touching a
    given row band form contiguous runs -> per-bucket matmuls
        psum[128ch, n_run] += RP[bucket][128 cells, ch]^T @ S[128, n_run]
    where S holds the 4 bilinear weights per sample (dense, bf16) and RP holds
    the PROJECTED grids (512 ch/cell: 256 for layer-0, 256 for layer-3).
  - Remaining MLP (h1, h2, w3h part, out) in bf16, weights stationary.
  - All matmul accumulation in fp32 PSUM; gelu + per-channel bias fused on the
    scalar engine; per-sample enc bias added on the vector engine.
"""

import numpy as np
import ml_dtypes

BF16 = ml_dtypes.bfloat16

B, H, W, C = 8, 64, 64, 256
N = 16384
NUM_FREQS = 10
MLP_WIDTH = 256
IN_DIM = 2 + 4 * NUM_FREQS + 3 * C  # 810

NSUP = 8            # column supers
SUP = N // NSUP     # 2048
NCH = 4             # 512-chunks per super
CH = 512

LEVEL_SIZES = [64, 32, 16]
# per-level k-layout of the RP (row-pair) stationary tensors:
#   L0: bucket g in [0,63): partitions r*64+x  = grid rows (g, g+1)
#   L1: bucket b in [0,11): partitions r*32+x  = grid rows (3b .. 3b+3)
#   L2: quad   q in [0,4):  partitions 32*rb + dy*16 + x = rows (4q+rb, 4q+rb+1)
N_BUCKETS = [63, 11, 4]
RPC = 512           # projected channels per grid cell (256 for L0-pass, 256 for L3-pass)


def _resize_matrix(out_size: int, in_size: int) -> np.ndarray:
    """Row-resize operator of jax.image.resize(..., 'bilinear') (antialias).
    Returns M [out_size, in_size] with resized = M @ x."""
    scale = out_size / in_size
    inv_scale = 1.0 / scale
    kernel_scale = max(inv_scale, 1.0)
    sample_f = (np.arange(out_size, dtype=np.float64) + 0.5) * inv_scale - 0.5
    x = np.abs(sample_f[None, :] - np.arange(in_size, dtype=np.float64)[:, None])
    x = x / kernel_scale
    w = np.where(x < 1.0, 1.0 - x, 0.0)
    total = w.sum(axis=0, keepdims=True)
    w = np.where(
        np.abs(total) > 1000.0 * np.finfo(np.float32).eps,
        w / np.where(total != 0.0, total, 1.0),
        0.0,
    )
    w = np.where(
        ((sample_f >= -0.5) & (sample_f <= in_size - 0.5))[None, :], w, 0.0
    )
    return w.T.astype(np.float32)  # [out, in]


def _posenc(coords: np.ndarray) -> np.ndarray:
    """Positional encoding [n, 42] fp32, matching reference order."""
    freqs = (2.0 ** np.arange(NUM_FREQS, dtype=np.float32)) * np.float32(np.pi)
    parts = [coords.astype(np.float32)]
    for f in freqs:
        parts.append(np.sin(coords * f).astype(np.float32))
        parts.append(np.cos(coords * f).astype(np.float32))
    return np.concatenate(parts, axis=1)  # [n, 42]


def _bilinear(c01: np.ndarray, size: int):
    """c01 [n] in [0,1] -> (i0, frac) fp32 like the reference's fp32 math."""
    cr = (c01 * np.float32(size - 1)).astype(np.float32)
    i0 = np.floor(cr).astype(np.int64)
    i0 = np.clip(i0, 0, size - 2)
    f = cr - i0.astype(np.float32)
    return i0, f.astype(np.float32)


def _host_prep(feature_grid, coords, w0, b0, w1, b1, w2, b2, w3, b3, w_out, b_out):
    """All host-side packing. Returns (shared_map, per_core_maps, perm, runs)."""
    fg = np.asarray(feature_grid, dtype=np.float32)
    coords = np.asarray(coords, dtype=np.float32)

    # ---- sort samples by continuous y so every level's y-buckets are runs ----
    c01 = (coords + np.float32(1.0)) / np.float32(2.0)  # [N,2] (y, x)
    perm = np.argsort(c01[:, 0], kind="stable")
    c01s = c01[perm]
    coords_s = coords[perm]

    # ---- per-level bilinear indices / weights / buckets -----------------------
    y0, fy, x0, fx, buckets = [], [], [], [], []
    for li, S in enumerate(LEVEL_SIZES):
        yi, fyi = _bilinear(c01s[:, 0], S)
        xi, fxi = _bilinear(c01s[:, 1], S)
        y0.append(yi); fy.append(fyi); x0.append(xi); fx.append(fxi)
        if li == 0:
            buckets.append(yi.copy())
        elif li == 1:
            buckets.append(yi // 3)
        else:
            buckets.append(yi // 4)

    # ---- dense S^T matrices [128, N] bf16 ------------------------------------
    s_t = []
    for li in range(3):
        Sm = np.zeros((N, 128), np.float32)
        wtl = (1 - fy[li]) * (1 - fx[li])
        wtr = (1 - fy[li]) * fx[li]
        wbl = fy[li] * (1 - fx[li])
        wbr = fy[li] * fx[li]
        j = np.arange(N)
        if li == 0:
            ktop = x0[li]
            kbot = 64 + x0[li]
        elif li == 1:
            dy_loc = y0[li] - 3 * buckets[li]
            ktop = dy_loc * 32 + x0[li]
            kbot = (dy_loc + 1) * 32 + x0[li]
        else:
            rb = y0[li] - 4 * buckets[li]
            ktop = rb * 32 + x0[li]
            kbot = rb * 32 + 16 + x0[li]
        Sm[j, ktop] = wtl
        Sm[j, ktop + 1] = wtr
        Sm[j, kbot] = wbl
        Sm[j, kbot + 1] = wbr
        s_t.append(np.ascontiguousarray(Sm.T).astype(BF16))

    # ---- bucket runs, split at CH boundaries ---------------------------------
    runs = []  # runs[level][chunk] = list of (bucket, off_in_chunk, length)
    for li in range(3):
        bk = buckets[li]
        per_chunk = [[] for _ in range(N // CH)]
        start = 0
        while start < N:
            g = bk[start]
            end = start
            while end < N and bk[end] == g:
                end += 1
            p = start
            while p < end:
                ci = p // CH
                q = min(end, (ci + 1) * CH)
                per_chunk[ci].append((int(g), p - ci * CH, q - p))
                p = q
            start = end
        runs.append(per_chunk)

    # ---- pyramid + weight-projected row-pair (RP) tensors per core -----------
    w0 = np.asarray(w0, np.float32); w1 = np.asarray(w1, np.float32)
    w2 = np.asarray(w2, np.float32); w3 = np.asarray(w3, np.float32)
    w_out = np.asarray(w_out, np.float32)

    R1 = _resize_matrix(32, 64)
    R2 = _resize_matrix(16, 64)
    g1 = np.einsum("ph,qw,bhwc->bpqc", R1, R1, fg, optimize=True)
    g2 = np.einsum("ph,qw,bhwc->bpqc", R2, R2, fg, optimize=True)
    pyr = [fg, g1, g2]

    # weight blocks: w0 rows [42:298 L0, 298:554 L1, 554:810 L2]
    #                w3 rows [0:256 h, 256:298 enc, 298:554 L0, 554:810 L1, 810:1066 L2]
    w0_l = [w0[42 + 256 * l: 42 + 256 * (l + 1)] for l in range(3)]   # [256,256]
    w3_l = [w3[298 + 256 * l: 298 + 256 * (l + 1)] for l in range(3)]

    # project each level's grid: proj[b, h, w, 0:256] = G @ w0_l, [256:512] = G @ w3_l
    proj = []
    for l in range(3):
        g = pyr[l]  # [B, h, w, 256]
        p0 = np.einsum("bhwc,cd->bhwd", g, w0_l[l], optimize=True)
        p3 = np.einsum("bhwc,cd->bhwd", g, w3_l[l], optimize=True)
        proj.append(np.concatenate([p0, p3], axis=-1).astype(np.float32))  # [B,h,w,512]

    def rp_tensors(p0b, p1b, p2b):
        # L0: [128, 63*512]: bucket g -> rows (g, g+1), partitions r*64+x
        rp0 = np.zeros((128, 63 * RPC), np.float32)
        for g in range(63):
            blk = p0b[g:g + 2]                      # [2, 64, 512]
            rp0[:, g * RPC:(g + 1) * RPC] = blk.reshape(128, RPC)
        # L1: [128, 11*512]: bucket b -> rows 3b..3b+3 (pad past row 31)
        rp1 = np.zeros((128, 11 * RPC), np.float32)
        for b in range(11):
            rows = p1b[3 * b:3 * b + 4]             # up to [4, 32, 512]
            blk = np.zeros((4, 32, RPC), np.float32)
            blk[:rows.shape[0]] = rows
            rp1[:, b * RPC:(b + 1) * RPC] = blk.reshape(128, RPC)
        # L2: [128, 4*512]: quad q, block rb -> rows (4q+rb, 4q+rb+1)
        rp2 = np.zeros((128, 4 * RPC), np.float32)
        for q in range(4):
            blk = np.zeros((4, 2, 16, RPC), np.float32)
            for rb in range(4):
                rows = p2b[4 * q + rb:4 * q + rb + 2]
                blk[rb, :rows.shape[0]] = rows
            rp2[:, q * RPC:(q + 1) * RPC] = blk.reshape(128, RPC)
        return rp0.astype(BF16), rp1.astype(BF16), rp2.astype(BF16)

    per_core = []
    for b in range(B):
        rp0, rp1, rp2 = rp_tensors(proj[0][b], proj[1][b], proj[2][b])
        per_core.append({"rp0": rp0, "rp1": rp1, "rp2": rp2})

    # ---- per-sample enc biases [128, 2, N] bf16 (m-tile on dim1) -------------
    enc = _posenc(coords_s)                            # [N, 42]
    bias0 = (enc @ w0[0:42]).T.astype(np.float32)      # [256, N]
    bias3 = (enc @ w3[256:298]).T.astype(np.float32)   # [256, N]
    bias0 = np.ascontiguousarray(bias0.reshape(2, 128, N).transpose(1, 0, 2)).astype(BF16)
    bias3 = np.ascontiguousarray(bias3.reshape(2, 128, N).transpose(1, 0, 2)).astype(BF16)

    def pack(wd):  # [256, M] -> [128, 2 * M], k-tile major on dim1
        K, M = wd.shape
        assert K == 256
        return np.ascontiguousarray(
            wd.reshape(2, 128, M).transpose(1, 0, 2).reshape(128, -1)
        )

    woutd = np.zeros((256, 3), np.float32)
    woutd[:] = w_out

    shared = {
        "s0t": s_t[0], "s1t": s_t[1], "s2t": s_t[2],
        "bias0": bias0.reshape(128, 2 * N), "bias3": bias3.reshape(128, 2 * N),
        "w1": pack(w1).astype(BF16), "w2": pack(w2).astype(BF16),
        "w3h": pack(w3[0:256]).astype(BF16),
        "wout": pack(woutd).astype(BF16),
        "b0": np.asarray(b0, np.float32).reshape(2, 128).T.copy(),
        "b1": np.asarray(b1, np.float32).reshape(2, 128).T.copy(),
        "b2": np.asarray(b2, np.float32).reshape(2, 128).T.copy(),
        "b3": np.asarray(b3, np.float32).reshape(2, 128).T.copy(),
        "bout": np.asarray(b_out, np.float32).reshape(3, 1).copy(),
    }
    return shared, per_core, perm, runs


_DRAM_SPECS = [
    # name, shape, np dtype
    ("rp0", (128, 63 * RPC), BF16),
    ("rp1", (128, 11 * RPC), BF16),
    ("rp2", (128, 4 * RPC), BF16),
    ("s0t", (128, N), BF16),
    ("s1t", (128, N), BF16),
    ("s2t", (128, N), BF16),
    ("bias0", (128, 2 * N), BF16),
    ("bias3", (128, 2 * N), BF16),
    ("w1", (128, 2 * 256), BF16),
    ("w2", (128, 2 * 256), BF16),
    ("w3h", (128, 2 * 256), BF16),
    ("wout", (128, 2 * 3), BF16),
    ("b0", (128, 2), np.float32),
    ("b1", (128, 2), np.float32),
    ("b2", (128, 2), np.float32),
    ("b3", (128, 2), np.float32),
    ("bout", (3, 1), np.float32),
]


def _build_nc(runs):
    """Build the Bacc program (shared by all cores; per-core data differs)."""
    from contextlib import ExitStack

    import concourse.bacc as bacc
    import concourse.mybir as mybir
    import concourse.tile as tile

    bf16 = mybir.dt.bfloat16
    f32 = mybir.dt.float32
    GELU = mybir.ActivationFunctionType.Gelu_apprx_tanh
    TANH = mybir.ActivationFunctionType.Tanh

    nc = bacc.Bacc("TRN2", debug=False, target_bir_lowering=False)

    dram = {}
    for name, shape, npdt in _DRAM_SPECS:
        dram[name] = nc.dram_tensor(
            name, list(shape), mybir.dt.from_np(np.dtype(npdt)), kind="ExternalInput"
        )
    out_dram = nc.dram_tensor("out_t", [3, N], f32, kind="ExternalOutput")

    with tile.TileContext(nc) as tc, ExitStack() as ctx:
        const = ctx.enter_context(tc.tile_pool(name="const", bufs=1))
        spool = ctx.enter_context(tc.tile_pool(name="stream", bufs=2))
        bpool = ctx.enter_context(tc.tile_pool(name="biasstream", bufs=1))
        hpool = ctx.enter_context(tc.tile_pool(name="h", bufs=2))
        opool = ctx.enter_context(tc.tile_pool(name="osb", bufs=2))
        ps_samp = ctx.enter_context(tc.tile_pool(name="ps_samp", bufs=3, space="PSUM"))
        ps_mlp = ctx.enter_context(tc.tile_pool(name="ps_mlp", bufs=4, space="PSUM"))
        ps_out = ctx.enter_context(tc.tile_pool(name="ps_out", bufs=1, space="PSUM"))

        # ---- static tensors ---------------------------------------------------
        st = {}
        # load order matters: small rp tensors first so sampling (L2, L1)
        # can start while the 8MB rp0 is still in flight; rp0 is split into
        # 8 independent loads so low buckets unblock early.
        order = ["rp2", "rp1", "w1", "w2", "w3h", "wout",
                 "b0", "b1", "b2", "b3", "bout", "rp0"]
        specs = {n: (s, d) for n, s, d in _DRAM_SPECS}
        for name in order:
            shape, npdt = specs[name]
            t = const.tile(list(shape), mybir.dt.from_np(np.dtype(npdt)), tag=name)
            if name == "rp0":
                q = shape[1] // 8
                for i in range(8):
                    nc.sync.dma_start(t[:, i * q:(i + 1) * q],
                                      dram[name][:, i * q:(i + 1) * q])
            else:
                nc.sync.dma_start(t[:, :], dram[name][:, :])
            st[name] = t

        rp = [st["rp0"], st["rp1"], st["rp2"]]
        wmlp = {"w1": st["w1"], "w2": st["w2"], "w3h": st["w3h"]}

        for s in range(NSUP):
            lo = s * SUP
            sl = slice(lo, lo + SUP)
            s_tiles = []
            for nm in ("s0t", "s1t", "s2t"):
                t = spool.tile([128, SUP], bf16, tag=nm)
                nc.sync.dma_start(t[:, :], dram[nm][:, sl])
                s_tiles.append(t)
            bias_t = {}
            for nm in ("bias0", "bias3"):
                t = bpool.tile([128, 2, SUP], bf16, tag=nm)
                # dram [128, 2*N]: columns m*N + n
                nc.sync.dma_start(t[:, 0, :], dram[nm][:, lo:lo + SUP])
                nc.sync.dma_start(t[:, 1, :], dram[nm][:, N + lo:N + lo + SUP])
                bias_t[nm] = t

            def sample_into(p, m, ch, pass_off, last_extra):
                """Accumulate the 3 pyramid levels' bucket-runs for chunk ch
                (columns of this super) into psum tile p, channel m-tile m.
                pass_off selects the layer-0 (0) or layer-3 (256) projection.
                last_extra: if True, leave the accumulation group open (caller
                adds more matmuls); else close it on the last L0 run."""
                for li in (2, 1, 0):
                    rlist = runs[li][s * NCH + ch]
                    for ri, (g, off, ln) in enumerate(rlist):
                        first = li == 2 and ri == 0
                        last = (not last_extra) and li == 0 and ri == len(rlist) - 1
                        col = g * RPC + pass_off + m * 128
                        nc.tensor.matmul(
                            p[:, off:off + ln],
                            rp[li][:, col:col + 128],
                            s_tiles[li][:, ch * CH + off: ch * CH + off + ln],
                            start=first, stop=last,
                        )

            # ---- pass 1: sampling -> +bias0 -> gelu -> h0 ---------------------
            h0 = hpool.tile([128, 2, SUP], bf16, tag="h0")
            for m in range(2):
                for ch in range(NCH):
                    p = ps_samp.tile([128, CH], f32, tag="ps_samp")
                    sample_into(p, m, ch, 0, last_extra=False)
                    nc.vector.tensor_add(
                        p[:, :], p[:, :], bias_t["bias0"][:, m, ch * CH:(ch + 1) * CH])
                    nc.scalar.activation(
                        h0[:, m, ch * CH:(ch + 1) * CH], p[:, :], GELU,
                        bias=st["b0"][:, m:m + 1])

            # ---- dense hidden layers ------------------------------------------
            def dense(wname, bname, rhs, tag):
                h = hpool.tile([128, 2, SUP], bf16, tag=tag)
                w = wmlp[wname]
                for m in range(2):
                    for ns in range(NCH):
                        p = ps_mlp.tile([128, CH], f32, tag="ps_mlp")
                        for kt in range(2):
                            nc.tensor.matmul(
                                p[:, :],
                                w[:, kt * 256 + m * 128: kt * 256 + m * 128 + 128],
                                rhs[:, kt, ns * CH:(ns + 1) * CH],
                                start=(kt == 0), stop=(kt == 1),
                            )
                        nc.scalar.activation(
                            h[:, m, ns * CH:(ns + 1) * CH], p[:, :], GELU,
                            bias=st[bname][:, m:m + 1])
                return h

            h1 = dense("w1", "b1", h0, "h1")
            h2 = dense("w2", "b2", h1, "h2")

            # ---- layer 3: sampling pass 2 + w3h @ h2 + bias3 -> gelu -> h3 ----
            # (reuses the h0 tag's buffers: h0 is dead once L1 is done)
            h3 = hpool.tile([128, 2, SUP], bf16, tag="h0", name="h3")
            for m in range(2):
                for ch in range(NCH):
                    p = ps_samp.tile([128, CH], f32, tag="ps_samp")
                    sample_into(p, m, ch, 256, last_extra=True)
                    for kt in range(2):
                        nc.tensor.matmul(
                            p[:, :],
                            st["w3h"][:, kt * 256 + m * 128: kt * 256 + m * 128 + 128],
                            h2[:, kt, ch * CH:(ch + 1) * CH],
                            start=False, stop=(kt == 1),
                        )
                    nc.vector.tensor_add(
                        p[:, :], p[:, :], bias_t["bias3"][:, m, ch * CH:(ch + 1) * CH])
                    nc.scalar.activation(
                        h3[:, m, ch * CH:(ch + 1) * CH], p[:, :], GELU,
                        bias=st["b3"][:, m:m + 1])

            # ---- output layer -------------------------------------------------
            osb = opool.tile([3, SUP], f32, tag="osb")
            for ns in range(NCH):
                po = ps_out.tile([128, CH], f32, tag="ps_out")
                for kt in range(2):
                    nc.tensor.matmul(
                        po[:3, :],
                        st["wout"][:, kt * 3:(kt + 1) * 3],
                        h3[:, kt, ns * CH:(ns + 1) * CH],
                        start=(kt == 0), stop=(kt == 1),
                    )
                nc.scalar.activation(
                    osb[:, ns * CH:(ns + 1) * CH], po[:3, :], TANH,
                    bias=st["bout"][:, 0:1],
                )
            nc.sync.dma_start(out_dram[:, sl], osb[:, :])

    nc.compile()
    return nc


def kernel(feature_grid, coords, w0, b0, w1, b1, w2, b2, w3, b3, w_out, b_out,
           _run_opts=None):
    from concourse.bass_utils import run_bass_kernel_spmd

    shared, per_core, perm, runs = _host_prep(
        feature_grid, coords, w0, b0, w1, b1, w2, b2, w3, b3, w_out, b_out)

    nc = _build_nc(runs)

    in_maps = []
    for b in range(B):
        m = dict(shared)
        m.update(per_core[b])
        in_maps.append(m)

    res = run_bass_kernel_spmd(
        nc, in_maps, core_ids=list(range(B)), **(_run_opts or {})
    )

    out = np.empty((B, N, 3), np.float32)
    inv = perm  # out_sorted column j corresponds to original sample perm[j]
    for b in range(B):
        out[b, inv, :] = res.results[b]["out_t"].T
    if _run_opts is not None:
        kernel._last_result = res  # for test harness introspection
    return out


# revision 7
# speedup vs baseline: 1.5357x; 1.1917x over previous
"""Trainium2 Bass kernel for nn_CoordinateDecoder (v2: projected-grid sampling).

Computation (see reference): posenc(coords) ++ bilinear-pyramid-sampled
features -> 5-layer MLP (gelu tanh-approx, skip concat at depth 2, tanh out).

Key idea v2: bilinear sampling is LINEAR in the grid, so
    w0_feat^T . bilerp(G_l, p) = bilerp(G_l @ w0_feat, p).
The host projects each pyramid level's grid through the layer-0 and layer-3
feature weight blocks; the device's sampling matmuls then directly produce the
layer-0 / layer-3 pre-activations in PSUM. This removes the two wide (k=810)
MLP matmuls entirely - they are replaced by a second sampling pass.
The posenc contribution becomes a host-computed per-sample bias [256, N]
added on the (otherwise idle) vector engine before each gelu.

  - Data-parallel over B: core b handles batch image b (coords/weights shared).
  - Samples host-sorted by continuous y; per pyramid level, samples touching a
    given row band form contiguous runs -> per-bucket matmuls
        psum[128ch, n_run] += RP[bucket][128 cells, ch]^T @ S[128, n_run]
    where S holds the 4 bilinear weights per sample (dense, bf16) and RP holds
    the PROJECTED grids (512 ch/cell: 256 for layer-0, 256 for layer-3).
  - Remaining MLP (h1, h2, w3h part, out) in bf16, weights stationary.
  - All matmul accumulation in fp32 PSUM; gelu + per-channel bias fused on the
    scalar engine; per-sample enc bias added on the vector engine.
"""

import numpy as np
import ml_dtypes

BF16 = ml_dtypes.bfloat16

B, H, W, C = 8, 64, 64, 256
N = 16384
NUM_FREQS = 10
MLP_WIDTH = 256
IN_DIM = 2 + 4 * NUM_FREQS + 3 * C  # 810

NSUP = 8            # column supers
SUP = N // NSUP     # 2048
NCH = 4             # 512-chunks per super
CH = 512

LEVEL_SIZES = [64, 32, 16]
# per-level k-layout of the RP (row-pair) stationary tensors:
#   L0: bucket g in [0,63): partitions r*64+x  = grid rows (g, g+1)
#   L1: bucket b in [0,11): partitions r*32+x  = grid rows (3b .. 3b+3)
#   L2: quad   q in [0,4):  partitions 32*rb + dy*16 + x = rows (4q+rb, 4q+rb+1)
N_BUCKETS = [63, 11, 4]
RPC = 512           # projected channels per grid cell (256 for L0-pass, 256 for L3-pass)


def _resize_matrix(out_size: int, in_size: int) -> np.ndarray:
    """Row-resize operator of jax.image.resize(..., 'bilinear') (antialias).
    Returns M [out_size, in_size] with resized = M @ x."""
    scale = out_size / in_size
    inv_scale = 1.0 / scale
    kernel_scale = max(inv_scale, 1.0)
    sample_f = (np.arange(out_size, dtype=np.float64) + 0.5) * inv_scale - 0.5
    x = np.abs(sample_f[None, :] - np.arange(in_size, dtype=np.float64)[:, None])
    x = x / kernel_scale
    w = np.where(x < 1.0, 1.0 - x, 0.0)
    total = w.sum(axis=0, keepdims=True)
    w = np.where(
        np.abs(total) > 1000.0 * np.finfo(np.float32).eps,
        w / np.where(total != 0.0, total, 1.0),
        0.0,
    )
    w = np.where(
        ((sample_f >= -0.5) & (sample_f <= in_size - 0.5))[None, :], w, 0.0
    )
    return w.T.astype(np.float32)  # [out, in]


def _posenc(coords: np.ndarray) -> np.ndarray:
    """Positional encoding [n, 42] fp32, matching reference order."""
    freqs = (2.0 ** np.arange(NUM_FREQS, dtype=np.float32)) * np.float32(np.pi)
    parts = [coords.astype(np.float32)]
    for f in freqs:
        parts.append(np.sin(coords * f).astype(np.float32))
        parts.append(np.cos(coords * f).astype(np.float32))
    return np.concatenate(parts, axis=1)  # [n, 42]


def _bilinear(c01: np.ndarray, size: int):
    """c01 [n] in [0,1] -> (i0, frac) fp32 like the reference's fp32 math."""
    cr = (c01 * np.float32(size - 1)).astype(np.float32)
    i0 = np.floor(cr).astype(np.int64)
    i0 = np.clip(i0, 0, size - 2)
    f = cr - i0.astype(np.float32)
    return i0, f.astype(np.float32)


def _host_prep(feature_grid, coords, w0, b0, w1, b1, w2, b2, w3, b3, w_out, b_out):
    """All host-side packing. Returns (shared_map, per_core_maps, perm, runs)."""
    fg = np.asarray(feature_grid, dtype=np.float32)
    coords = np.asarray(coords, dtype=np.float32)

    # ---- sort samples by continuous y so every level's y-buckets are runs ----
    c01 = (coords + np.float32(1.0)) / np.float32(2.0)  # [N,2] (y, x)
    perm = np.argsort(c01[:, 0], kind="stable")
    c01s = c01[perm]
    coords_s = coords[perm]

    # ---- per-level bilinear indices / weights / buckets -----------------------
    y0, fy, x0, fx, buckets = [], [], [], [], []
    for li, S in enumerate(LEVEL_SIZES):
        yi, fyi = _bilinear(c01s[:, 0], S)
        xi, fxi = _bilinear(c01s[:, 1], S)
        y0.append(yi); fy.append(fyi); x0.append(xi); fx.append(fxi)
        if li == 0:
            buckets.append(yi.copy())
        elif li == 1:
            buckets.append(yi // 3)
        else:
            buckets.append(yi // 4)

    # ---- dense S^T matrices [128, N] bf16 ------------------------------------
    s_t = []
    for li in range(3):
        Sm = np.zeros((N, 128), np.float32)
        wtl = (1 - fy[li]) * (1 - fx[li])
        wtr = (1 - fy[li]) * fx[li]
        wbl = fy[li] * (1 - fx[li])
        wbr = fy[li] * fx[li]
        j = np.arange(N)
        if li == 0:
            ktop = x0[li]
            kbot = 64 + x0[li]
        elif li == 1:
            dy_loc = y0[li] - 3 * buckets[li]
            ktop = dy_loc * 32 + x0[li]
            kbot = (dy_loc + 1) * 32 + x0[li]
        else:
            rb = y0[li] - 4 * buckets[li]
            ktop = rb * 32 + x0[li]
            kbot = rb * 32 + 16 + x0[li]
        Sm[j, ktop] = wtl
        Sm[j, ktop + 1] = wtr
        Sm[j, kbot] = wbl
        Sm[j, kbot + 1] = wbr
        s_t.append(np.ascontiguousarray(Sm.T).astype(BF16))

    # ---- bucket runs, split at CH boundaries ---------------------------------
    runs = []  # runs[level][chunk] = list of (bucket, off_in_chunk, length)
    for li in range(3):
        bk = buckets[li]
        per_chunk = [[] for _ in range(N // CH)]
        start = 0
        while start < N:
            g = bk[start]
            end = start
            while end < N and bk[end] == g:
                end += 1
            p = start
            while p < end:
                ci = p // CH
                q = min(end, (ci + 1) * CH)
                per_chunk[ci].append((int(g), p - ci * CH, q - p))
                p = q
            start = end
        runs.append(per_chunk)

    # ---- pyramid + weight-projected row-pair (RP) tensors per core -----------
    w0 = np.asarray(w0, np.float32); w1 = np.asarray(w1, np.float32)
    w2 = np.asarray(w2, np.float32); w3 = np.asarray(w3, np.float32)
    w_out = np.asarray(w_out, np.float32)

    R1 = _resize_matrix(32, 64)
    R2 = _resize_matrix(16, 64)
    g1 = np.einsum("ph,qw,bhwc->bpqc", R1, R1, fg, optimize=True)
    g2 = np.einsum("ph,qw,bhwc->bpqc", R2, R2, fg, optimize=True)
    pyr = [fg, g1, g2]

    # weight blocks: w0 rows [42:298 L0, 298:554 L1, 554:810 L2]
    #                w3 rows [0:256 h, 256:298 enc, 298:554 L0, 554:810 L1, 810:1066 L2]
    w0_l = [w0[42 + 256 * l: 42 + 256 * (l + 1)] for l in range(3)]   # [256,256]
    w3_l = [w3[298 + 256 * l: 298 + 256 * (l + 1)] for l in range(3)]

    # project each level's grid: proj[b, h, w, 0:256] = G @ w0_l, [256:512] = G @ w3_l
    proj = []
    for l in range(3):
        g = pyr[l]  # [B, h, w, 256]
        p0 = np.einsum("bhwc,cd->bhwd", g, w0_l[l], optimize=True)
        p3 = np.einsum("bhwc,cd->bhwd", g, w3_l[l], optimize=True)
        proj.append(np.concatenate([p0, p3], axis=-1).astype(np.float32))  # [B,h,w,512]

    def rp_tensors(p0b, p1b, p2b):
        # L0: [128, 63*512]: bucket g -> rows (g, g+1), partitions r*64+x
        rp0 = np.zeros((128, 63 * RPC), np.float32)
        for g in range(63):
            blk = p0b[g:g + 2]                      # [2, 64, 512]
            rp0[:, g * RPC:(g + 1) * RPC] = blk.reshape(128, RPC)
        # L1: [128, 11*512]: bucket b -> rows 3b..3b+3 (pad past row 31)
        rp1 = np.zeros((128, 11 * RPC), np.float32)
        for b in range(11):
            rows = p1b[3 * b:3 * b + 4]             # up to [4, 32, 512]
            blk = np.zeros((4, 32, RPC), np.float32)
            blk[:rows.shape[0]] = rows
            rp1[:, b * RPC:(b + 1) * RPC] = blk.reshape(128, RPC)
        # L2: [128, 4*512]: quad q, block rb -> rows (4q+rb, 4q+rb+1)
        rp2 = np.zeros((128, 4 * RPC), np.float32)
        for q in range(4):
            blk = np.zeros((4, 2, 16, RPC), np.float32)
            for rb in range(4):
                rows = p2b[4 * q + rb:4 * q + rb + 2]
                blk[rb, :rows.shape[0]] = rows
            rp2[:, q * RPC:(q + 1) * RPC] = blk.reshape(128, RPC)
        return rp0.astype(BF16), rp1.astype(BF16), rp2.astype(BF16)

    per_core = []
    for b in range(B):
        rp0, rp1, rp2 = rp_tensors(proj[0][b], proj[1][b], proj[2][b])
        per_core.append({"rp0": rp0, "rp1": rp1, "rp2": rp2})

    # ---- per-sample enc biases [128, 2, N] bf16 (m-tile on dim1) -------------
    enc = _posenc(coords_s)                            # [N, 42]
    bias0 = (enc @ w0[0:42]).T.astype(np.float32)      # [256, N]
    bias3 = (enc @ w3[256:298]).T.astype(np.float32)   # [256, N]
    bias0 = np.ascontiguousarray(bias0.reshape(2, 128, N).transpose(1, 0, 2)).astype(BF16)
    bias3 = np.ascontiguousarray(bias3.reshape(2, 128, N).transpose(1, 0, 2)).astype(BF16)

    def pack(wd):  # [256, M] -> [128, 2 * M], k-tile major on dim1
        K, M = wd.shape
        assert K == 256
        return np.ascontiguousarray(
            wd.reshape(2, 128, M).transpose(1, 0, 2).reshape(128, -1)
        )

    woutd = np.zeros((256, 3), np.float32)
    woutd[:] = w_out

    shared = {
        "s0t": s_t[0], "s1t": s_t[1], "s2t": s_t[2],
        "bias0": bias0.reshape(128, 2 * N), "bias3": bias3.reshape(128, 2 * N),
        "w1": pack(w1).astype(BF16), "w2": pack(w2).astype(BF16),
        "w3h": pack(w3[0:256]).astype(BF16),
        "wout": pack(woutd).astype(BF16),
        "b0": np.asarray(b0, np.float32).reshape(2, 128).T.copy(),
        "b1": np.asarray(b1, np.float32).reshape(2, 128).T.copy(),
        "b2": np.asarray(b2, np.float32).reshape(2, 128).T.copy(),
        "b3": np.asarray(b3, np.float32).reshape(2, 128).T.copy(),
        "bout": np.asarray(b_out, np.float32).reshape(3, 1).copy(),
    }
    return shared, per_core, perm, runs


_DRAM_SPECS = [
    # name, shape, np dtype
    ("rp0", (128, 63 * RPC), BF16),
    ("rp1", (128, 11 * RPC), BF16),
    ("rp2", (128, 4 * RPC), BF16),
    ("s0t", (128, N), BF16),
    ("s1t", (128, N), BF16),
    ("s2t", (128, N), BF16),
    ("bias0", (128, 2 * N), BF16),
    ("bias3", (128, 2 * N), BF16),
    ("w1", (128, 2 * 256), BF16),
    ("w2", (128, 2 * 256), BF16),
    ("w3h", (128, 2 * 256), BF16),
    ("wout", (128, 2 * 3), BF16),
    ("b0", (128, 2), np.float32),
    ("b1", (128, 2), np.float32),
    ("b2", (128, 2), np.float32),
    ("b3", (128, 2), np.float32),
    ("bout", (3, 1), np.float32),
]


def _build_nc(runs):
    """Build the Bacc program (shared by all cores; per-core data differs)."""
    from contextlib import ExitStack

    import concourse.bacc as bacc
    import concourse.mybir as mybir
    import concourse.tile as tile

    bf16 = mybir.dt.bfloat16
    f32 = mybir.dt.float32
    GELU = mybir.ActivationFunctionType.Gelu_apprx_tanh
    TANH = mybir.ActivationFunctionType.Tanh

    nc = bacc.Bacc("TRN2", debug=False, target_bir_lowering=False)

    dram = {}
    for name, shape, npdt in _DRAM_SPECS:
        dram[name] = nc.dram_tensor(
            name, list(shape), mybir.dt.from_np(np.dtype(npdt)), kind="ExternalInput"
        )
    out_dram = nc.dram_tensor("out_t", [3, N], f32, kind="ExternalOutput")

    with tile.TileContext(nc) as tc, ExitStack() as ctx:
        const = ctx.enter_context(tc.tile_pool(name="const", bufs=1))
        spool = ctx.enter_context(tc.tile_pool(name="stream", bufs=2))
        bpool = ctx.enter_context(tc.tile_pool(name="biasstream", bufs=2))
        hpool = ctx.enter_context(tc.tile_pool(name="h", bufs=2))
        opool = ctx.enter_context(tc.tile_pool(name="osb", bufs=2))
        ps_samp = ctx.enter_context(tc.tile_pool(name="ps_samp", bufs=3, space="PSUM"))
        ps_mlp = ctx.enter_context(tc.tile_pool(name="ps_mlp", bufs=4, space="PSUM"))
        ps_out = ctx.enter_context(tc.tile_pool(name="ps_out", bufs=1, space="PSUM"))

        # ---- static tensors ---------------------------------------------------
        st = {}
        # load order matters: small rp tensors + weights first, then the first
        # two supers' streaming tiles, and only THEN the 8MB rp0 (split into 8
        # independent loads so low buckets unblock early) - this way sampling
        # can begin a few us in rather than waiting behind rp0.
        order = ["rp2", "rp1", "w1", "w2", "w3h", "wout",
                 "b0", "b1", "b2", "b3", "bout"]
        specs = {n: (s, d) for n, s, d in _DRAM_SPECS}
        for name in order:
            shape, npdt = specs[name]
            t = const.tile(list(shape), mybir.dt.from_np(np.dtype(npdt)), tag=name)
            nc.sync.dma_start(t[:, :], dram[name][:, :])
            st[name] = t

        stream_tiles = {}

        def issue_stream(s):
            lo = s * SUP
            s_tiles = []
            for nm in ("s0t", "s1t", "s2t"):
                t = spool.tile([128, SUP], bf16, tag=nm, name=f"{nm}_{s}")
                nc.sync.dma_start(t[:, :], dram[nm][:, lo:lo + SUP])
                s_tiles.append(t)
            bias_t = {}
            for nm in ("bias0", "bias3"):
                t = bpool.tile([128, 2, SUP], bf16, tag=nm, name=f"{nm}_{s}")
                # dram [128, 2*N]: columns m*N + n
                nc.sync.dma_start(t[:, 0, :], dram[nm][:, lo:lo + SUP])
                nc.sync.dma_start(t[:, 1, :], dram[nm][:, N + lo:N + lo + SUP])
                bias_t[nm] = t
            stream_tiles[s] = (s_tiles, bias_t)

        issue_stream(0)
        issue_stream(1)

        shape, npdt = specs["rp0"]
        t = const.tile(list(shape), mybir.dt.from_np(np.dtype(npdt)), tag="rp0")
        q = shape[1] // 8
        for i in range(8):
            nc.sync.dma_start(t[:, i * q:(i + 1) * q],
                              dram["rp0"][:, i * q:(i + 1) * q])
        st["rp0"] = t

        rp = [st["rp0"], st["rp1"], st["rp2"]]
        wmlp = {"w1": st["w1"], "w2": st["w2"], "w3h": st["w3h"]}

        for s in range(NSUP):
            lo = s * SUP
            sl = slice(lo, lo + SUP)
            if s not in stream_tiles:
                issue_stream(s)
            s_tiles, bias_t = stream_tiles.pop(s)
            if s + 1 < NSUP and s + 1 not in stream_tiles:
                issue_stream(s + 1)  # a full super of DMA prefetch lead

            def sample_into(p, m, ch, pass_off, last_extra):
                """Accumulate the 3 pyramid levels' bucket-runs for chunk ch
                (columns of this super) into psum tile p, channel m-tile m.
                pass_off selects the layer-0 (0) or layer-3 (256) projection.
                last_extra: if True, leave the accumulation group open (caller
                adds more matmuls); else close it on the last L0 run."""
                for li in (2, 1, 0):
                    rlist = runs[li][s * NCH + ch]
                    for ri, (g, off, ln) in enumerate(rlist):
                        first = li == 2 and ri == 0
                        last = (not last_extra) and li == 0 and ri == len(rlist) - 1
                        col = g * RPC + pass_off + m * 128
                        nc.tensor.matmul(
                            p[:, off:off + ln],
                            rp[li][:, col:col + 128],
                            s_tiles[li][:, ch * CH + off: ch * CH + off + ln],
                            start=first, stop=last,
                        )

            # ---- pass 1: sampling -> +bias0 -> gelu -> h0 ---------------------
            h0 = hpool.tile([128, 2, SUP], bf16, tag="h0")
            for m in range(2):
                for ch in range(NCH):
                    p = ps_samp.tile([128, CH], f32, tag="ps_samp")
                    sample_into(p, m, ch, 0, last_extra=False)
                    nc.vector.tensor_add(
                        p[:, :], p[:, :], bias_t["bias0"][:, m, ch * CH:(ch + 1) * CH])
                    nc.scalar.activation(
                        h0[:, m, ch * CH:(ch + 1) * CH], p[:, :], GELU,
                        bias=st["b0"][:, m:m + 1])

            # ---- dense hidden layers ------------------------------------------
            def dense(wname, bname, rhs, tag):
                h = hpool.tile([128, 2, SUP], bf16, tag=tag)
                w = wmlp[wname]
                for m in range(2):
                    for ns in range(NCH):
                        p = ps_mlp.tile([128, CH], f32, tag="ps_mlp")
                        for kt in range(2):
                            nc.tensor.matmul(
                                p[:, :],
                                w[:, kt * 256 + m * 128: kt * 256 + m * 128 + 128],
                                rhs[:, kt, ns * CH:(ns + 1) * CH],
                                start=(kt == 0), stop=(kt == 1),
                            )
                        nc.scalar.activation(
                            h[:, m, ns * CH:(ns + 1) * CH], p[:, :], GELU,
                            bias=st[bname][:, m:m + 1])
                return h

            h1 = dense("w1", "b1", h0, "h1")
            h2 = dense("w2", "b2", h1, "h2")

            # ---- layer 3: sampling pass 2 + w3h @ h2 + bias3 -> gelu -> h3 ----
            # (reuses the h0 tag's buffers: h0 is dead once L1 is done)
            h3 = hpool.tile([128, 2, SUP], bf16, tag="h0", name="h3")
            for m in range(2):
                for ch in range(NCH):
                    p = ps_samp.tile([128, CH], f32, tag="ps_samp")
                    sample_into(p, m, ch, 256, last_extra=True)
                    for kt in range(2):
                        nc.tensor.matmul(
                            p[:, :],
                            st["w3h"][:, kt * 256 + m * 128: kt * 256 + m * 128 + 128],
                            h2[:, kt, ch * CH:(ch + 1) * CH],
                            start=False, stop=(kt == 1),
                        )
                    nc.vector.tensor_add(
                        p[:, :], p[:, :], bias_t["bias3"][:, m, ch * CH:(ch + 1) * CH])
                    nc.scalar.activation(
                        h3[:, m, ch * CH:(ch + 1) * CH], p[:, :], GELU,
                        bias=st["b3"][:, m:m + 1])

            # ---- output layer -------------------------------------------------
            osb = opool.tile([3, SUP], f32, tag="osb")
            for ns in range(NCH):
                po = ps_out.tile([128, CH], f32, tag="ps_out")
                for kt in range(2):
                    nc.tensor.matmul(
                        po[:3, :],
                        st["wout"][:, kt * 3:(kt + 1) * 3],
                        h3[:, kt, ns * CH:(ns + 1) * CH],
                        start=(kt == 0), stop=(kt == 1),
                    )
                nc.scalar.activation(
                    osb[:, ns * CH:(ns + 1) * CH], po[:3, :], TANH,
                    bias=st["bout"][:, 0:1],
                )
            nc.sync.dma_start(out_dram[:, sl], osb[:, :])

    nc.compile()
    return nc


def kernel(feature_grid, coords, w0, b0, w1, b1, w2, b2, w3, b3, w_out, b_out,
           _run_opts=None):
    from concourse.bass_utils import run_bass_kernel_spmd

    shared, per_core, perm, runs = _host_prep(
        feature_grid, coords, w0, b0, w1, b1, w2, b2, w3, b3, w_out, b_out)

    nc = _build_nc(runs)

    in_maps = []
    for b in range(B):
        m = dict(shared)
        m.update(per_core[b])
        in_maps.append(m)

    res = run_bass_kernel_spmd(
        nc, in_maps, core_ids=list(range(B)), **(_run_opts or {})
    )

    out = np.empty((B, N, 3), np.float32)
    inv = perm  # out_sorted column j corresponds to original sample perm[j]
    for b in range(B):
        out[b, inv, :] = res.results[b]["out_t"].T
    if _run_opts is not None:
        kernel._last_result = res  # for test harness introspection
    return out


# revision 17
# speedup vs baseline: 1.5699x; 1.0223x over previous
"""Trainium2 Bass kernel for nn_CoordinateDecoder (v2: projected-grid sampling).

Computation (see reference): posenc(coords) ++ bilinear-pyramid-sampled
features -> 5-layer MLP (gelu tanh-approx, skip concat at depth 2, tanh out).

Key idea v2: bilinear sampling is LINEAR in the grid, so
    w0_feat^T . bilerp(G_l, p) = bilerp(G_l @ w0_feat, p).
The host projects each pyramid level's grid through the layer-0 and layer-3
feature weight blocks; the device's sampling matmuls then directly produce the
layer-0 / layer-3 pre-activations in PSUM. This removes the two wide (k=810)
MLP matmuls entirely - they are replaced by a second sampling pass.
The posenc contribution becomes a host-computed per-sample bias [256, N]
added on the (otherwise idle) vector engine before each gelu.

  - Data-parallel over B: core b handles batch image b (coords/weights shared).
  - Samples host-sorted by continuous y; per pyramid level, samples touching a
    given row band form contiguous runs -> per-bucket matmuls
        psum[128ch, n_run] += RP[bucket][128 cells, ch]^T @ S[128, n_run]
    where S holds the 4 bilinear weights per sample (dense, bf16) and RP holds
    the PROJECTED grids (512 ch/cell: 256 for layer-0, 256 for layer-3).
  - Remaining MLP (h1, h2, w3h part, out) in bf16, weights stationary.
  - All matmul accumulation in fp32 PSUM; gelu + per-channel bias fused on the
    scalar engine; per-sample enc bias added on the vector engine.
"""

import numpy as np
import ml_dtypes

BF16 = ml_dtypes.bfloat16
F8E4 = ml_dtypes.float8_e4m3  # trn FP8_EXP4-compatible (max +-240)

B, H, W, C = 8, 64, 64, 256
N = 16384
NUM_FREQS = 10
MLP_WIDTH = 256
IN_DIM = 2 + 4 * NUM_FREQS + 3 * C  # 810

NSUP = 8            # column supers
SUP = N // NSUP     # 2048
NCH = 4             # 512-chunks per super
CH = 512

LEVEL_SIZES = [64, 32, 16]
# per-level k-layout of the RP (row-pair) stationary tensors:
#   L0: bucket g in [0,63): partitions r*64+x  = grid rows (g, g+1)
#   L1: bucket b in [0,11): partitions r*32+x  = grid rows (3b .. 3b+3)
#   L2: quad   q in [0,4):  partitions 32*rb + dy*16 + x = rows (4q+rb, 4q+rb+1)
N_BUCKETS = [63, 11, 4]
RPC = 512           # projected channels per grid cell (256 for L0-pass, 256 for L3-pass)


def _resize_matrix(out_size: int, in_size: int) -> np.ndarray:
    """Row-resize operator of jax.image.resize(..., 'bilinear') (antialias).
    Returns M [out_size, in_size] with resized = M @ x."""
    scale = out_size / in_size
    inv_scale = 1.0 / scale
    kernel_scale = max(inv_scale, 1.0)
    sample_f = (np.arange(out_size, dtype=np.float64) + 0.5) * inv_scale - 0.5
    x = np.abs(sample_f[None, :] - np.arange(in_size, dtype=np.float64)[:, None])
    x = x / kernel_scale
    w = np.where(x < 1.0, 1.0 - x, 0.0)
    total = w.sum(axis=0, keepdims=True)
    w = np.where(
        np.abs(total) > 1000.0 * np.finfo(np.float32).eps,
        w / np.where(total != 0.0, total, 1.0),
        0.0,
    )
    w = np.where(
        ((sample_f >= -0.5) & (sample_f <= in_size - 0.5))[None, :], w, 0.0
    )
    return w.T.astype(np.float32)  # [out, in]


def _posenc(coords: np.ndarray) -> np.ndarray:
    """Positional encoding [n, 42] fp32, matching reference order."""
    freqs = (2.0 ** np.arange(NUM_FREQS, dtype=np.float32)) * np.float32(np.pi)
    parts = [coords.astype(np.float32)]
    for f in freqs:
        parts.append(np.sin(coords * f).astype(np.float32))
        parts.append(np.cos(coords * f).astype(np.float32))
    return np.concatenate(parts, axis=1)  # [n, 42]


def _bilinear(c01: np.ndarray, size: int):
    """c01 [n] in [0,1] -> (i0, frac) fp32 like the reference's fp32 math."""
    cr = (c01 * np.float32(size - 1)).astype(np.float32)
    i0 = np.floor(cr).astype(np.int64)
    i0 = np.clip(i0, 0, size - 2)
    f = cr - i0.astype(np.float32)
    return i0, f.astype(np.float32)


def _host_prep(feature_grid, coords, w0, b0, w1, b1, w2, b2, w3, b3, w_out, b_out):
    """All host-side packing. Returns (shared_map, per_core_maps, perm, runs)."""
    fg = np.asarray(feature_grid, dtype=np.float32)
    coords = np.asarray(coords, dtype=np.float32)

    # ---- sort samples by continuous y so every level's y-buckets are runs ----
    c01 = (coords + np.float32(1.0)) / np.float32(2.0)  # [N,2] (y, x)
    perm = np.argsort(c01[:, 0], kind="stable")
    c01s = c01[perm]
    coords_s = coords[perm]

    # ---- per-level bilinear indices / weights / buckets -----------------------
    y0, fy, x0, fx, buckets = [], [], [], [], []
    for li, S in enumerate(LEVEL_SIZES):
        yi, fyi = _bilinear(c01s[:, 0], S)
        xi, fxi = _bilinear(c01s[:, 1], S)
        y0.append(yi); fy.append(fyi); x0.append(xi); fx.append(fxi)
        if li == 0:
            buckets.append(yi.copy())
        elif li == 1:
            buckets.append(yi // 3)
        else:
            buckets.append(yi // 4)

    # ---- dense S^T matrices [128, N] bf16 ------------------------------------
    s_t = []
    for li in range(3):
        Sm = np.zeros((N, 128), np.float32)
        wtl = (1 - fy[li]) * (1 - fx[li])
        wtr = (1 - fy[li]) * fx[li]
        wbl = fy[li] * (1 - fx[li])
        wbr = fy[li] * fx[li]
        j = np.arange(N)
        if li == 0:
            ktop = x0[li]
            kbot = 64 + x0[li]
        elif li == 1:
            dy_loc = y0[li] - 3 * buckets[li]
            ktop = dy_loc * 32 + x0[li]
            kbot = (dy_loc + 1) * 32 + x0[li]
        else:
            rb = y0[li] - 4 * buckets[li]
            ktop = rb * 32 + x0[li]
            kbot = rb * 32 + 16 + x0[li]
        Sm[j, ktop] = wtl
        Sm[j, ktop + 1] = wtr
        Sm[j, kbot] = wbl
        Sm[j, kbot + 1] = wbr
        s_t.append(np.ascontiguousarray(Sm.T).astype(BF16))

    # ---- bucket runs, split at CH boundaries ---------------------------------
    runs = []  # runs[level][chunk] = list of (bucket, off_in_chunk, length)
    for li in range(3):
        bk = buckets[li]
        per_chunk = [[] for _ in range(N // CH)]
        start = 0
        while start < N:
            g = bk[start]
            end = start
            while end < N and bk[end] == g:
                end += 1
            p = start
            while p < end:
                ci = p // CH
                q = min(end, (ci + 1) * CH)
                per_chunk[ci].append((int(g), p - ci * CH, q - p))
                p = q
            start = end
        runs.append(per_chunk)

    # ---- pyramid + weight-projected row-pair (RP) tensors per core -----------
    w0 = np.asarray(w0, np.float32); w1 = np.asarray(w1, np.float32)
    w2 = np.asarray(w2, np.float32); w3 = np.asarray(w3, np.float32)
    w_out = np.asarray(w_out, np.float32)

    R1 = _resize_matrix(32, 64)
    R2 = _resize_matrix(16, 64)
    g1 = np.einsum("ph,qw,bhwc->bpqc", R1, R1, fg, optimize=True)
    g2 = np.einsum("ph,qw,bhwc->bpqc", R2, R2, fg, optimize=True)
    pyr = [fg, g1, g2]

    # weight blocks: w0 rows [42:298 L0, 298:554 L1, 554:810 L2]
    #                w3 rows [0:256 h, 256:298 enc, 298:554 L0, 554:810 L1, 810:1066 L2]
    w0_l = [w0[42 + 256 * l: 42 + 256 * (l + 1)] for l in range(3)]   # [256,256]
    w3_l = [w3[298 + 256 * l: 298 + 256 * (l + 1)] for l in range(3)]

    # project each level's grid: proj[b, h, w, 0:256] = G @ w0_l, [256:512] = G @ w3_l
    proj = []
    for l in range(3):
        g = pyr[l]  # [B, h, w, 256]
        p0 = np.einsum("bhwc,cd->bhwd", g, w0_l[l], optimize=True)
        p3 = np.einsum("bhwc,cd->bhwd", g, w3_l[l], optimize=True)
        proj.append(np.concatenate([p0, p3], axis=-1).astype(np.float32))  # [B,h,w,512]

    def rp_tensors(p0b, p1b, p2b):
        # L0: [128, 63*512]: bucket g -> rows (g, g+1), partitions r*64+x
        rp0 = np.zeros((128, 63 * RPC), np.float32)
        for g in range(63):
            blk = p0b[g:g + 2]                      # [2, 64, 512]
            rp0[:, g * RPC:(g + 1) * RPC] = blk.reshape(128, RPC)
        # L1: [128, 11*512]: bucket b -> rows 3b..3b+3 (pad past row 31)
        rp1 = np.zeros((128, 11 * RPC), np.float32)
        for b in range(11):
            rows = p1b[3 * b:3 * b + 4]             # up to [4, 32, 512]
            blk = np.zeros((4, 32, RPC), np.float32)
            blk[:rows.shape[0]] = rows
            rp1[:, b * RPC:(b + 1) * RPC] = blk.reshape(128, RPC)
        # L2: [128, 4*512]: quad q, block rb -> rows (4q+rb, 4q+rb+1)
        rp2 = np.zeros((128, 4 * RPC), np.float32)
        for q in range(4):
            blk = np.zeros((4, 2, 16, RPC), np.float32)
            for rb in range(4):
                rows = p2b[4 * q + rb:4 * q + rb + 2]
                blk[rb, :rows.shape[0]] = rows
            rp2[:, q * RPC:(q + 1) * RPC] = blk.reshape(128, RPC)
        return rp0.astype(BF16), rp1.astype(BF16), rp2.astype(BF16)

    per_core = []
    for b in range(B):
        rp0, rp1, rp2 = rp_tensors(proj[0][b], proj[1][b], proj[2][b])
        per_core.append({"rp0": rp0, "rp1": rp1, "rp2": rp2})

    # ---- per-sample enc biases [128, 2, N] bf16 (m-tile on dim1) -------------
    enc = _posenc(coords_s)                            # [N, 42]
    bias0 = (enc @ w0[0:42]).T.astype(np.float32)      # [256, N]
    bias3 = (enc @ w3[256:298]).T.astype(np.float32)   # [256, N]
    bias0 = np.ascontiguousarray(bias0.reshape(2, 128, N).transpose(1, 0, 2)).astype(BF16)
    bias3 = np.ascontiguousarray(bias3.reshape(2, 128, N).transpose(1, 0, 2)).astype(BF16)

    def pack(wd):  # [256, M] -> [128, 2, M], k-tile major on dim1
        K, M = wd.shape
        assert K == 256
        return np.ascontiguousarray(wd.reshape(2, 128, M).transpose(1, 0, 2))

    woutd = np.zeros((256, 3), np.float32)
    woutd[:] = w_out

    shared = {
        "s0t": s_t[0], "s1t": s_t[1], "s2t": s_t[2],
        "bias0": bias0.reshape(128, 2 * N), "bias3": bias3.reshape(128, 2 * N),
        # hidden-layer weights in fp8 e4m3 (used via DoubleRow matmuls);
        # the output head stays bf16 - fp8 there costs ~3.7e-2 rel err.
        "w1": pack(w1).astype(F8E4), "w2": pack(w2).astype(F8E4),
        "w3h": pack(w3[0:256]).astype(F8E4),
        "wout": pack(woutd).reshape(128, 2 * 3).astype(BF16),
        "b0": np.asarray(b0, np.float32).reshape(2, 128).T.copy(),
        "b1": np.asarray(b1, np.float32).reshape(2, 128).T.copy(),
        "b2": np.asarray(b2, np.float32).reshape(2, 128).T.copy(),
        "b3": np.asarray(b3, np.float32).reshape(2, 128).T.copy(),
        "bout": np.asarray(b_out, np.float32).reshape(3, 1).copy(),
    }
    return shared, per_core, perm, runs


_DRAM_SPECS = [
    # name, shape, np dtype
    ("rp0", (128, 63 * RPC), BF16),
    ("rp1", (128, 11 * RPC), BF16),
    ("rp2", (128, 4 * RPC), BF16),
    ("s0t", (128, N), BF16),
    ("s1t", (128, N), BF16),
    ("s2t", (128, N), BF16),
    ("bias0", (128, 2 * N), BF16),
    ("bias3", (128, 2 * N), BF16),
    ("w1", (128, 2, 256), F8E4),
    ("w2", (128, 2, 256), F8E4),
    ("w3h", (128, 2, 256), F8E4),
    ("wout", (128, 2 * 3), BF16),
    ("b0", (128, 2), np.float32),
    ("b1", (128, 2), np.float32),
    ("b2", (128, 2), np.float32),
    ("b3", (128, 2), np.float32),
    ("bout", (3, 1), np.float32),
]


def _build_nc(runs):
    """Build the Bacc program (shared by all cores; per-core data differs)."""
    from contextlib import ExitStack

    import concourse.bacc as bacc
    import concourse.mybir as mybir
    import concourse.tile as tile

    bf16 = mybir.dt.bfloat16
    f8e4 = mybir.dt.float8e4
    f32 = mybir.dt.float32
    GELU = mybir.ActivationFunctionType.Gelu_apprx_tanh
    TANH = mybir.ActivationFunctionType.Tanh
    DOUBLE_ROW = mybir.MatmulPerfMode.DoubleRow

    nc = bacc.Bacc("TRN2", debug=False, target_bir_lowering=False)

    dram = {}
    for name, shape, npdt in _DRAM_SPECS:
        dram[name] = nc.dram_tensor(
            name, list(shape), mybir.dt.from_np(np.dtype(npdt)), kind="ExternalInput"
        )
    out_dram = nc.dram_tensor("out_t", [3, N], f32, kind="ExternalOutput")

    with tile.TileContext(nc) as tc, ExitStack() as ctx:
        const = ctx.enter_context(tc.tile_pool(name="const", bufs=1))
        spool = ctx.enter_context(tc.tile_pool(name="stream", bufs=2))
        bpool = ctx.enter_context(tc.tile_pool(name="biasstream", bufs=2))
        hpool = ctx.enter_context(tc.tile_pool(name="h", bufs=2))
        opool = ctx.enter_context(tc.tile_pool(name="osb", bufs=2))
        ps_samp = ctx.enter_context(tc.tile_pool(name="ps_samp", bufs=3, space="PSUM"))
        ps_mlp = ctx.enter_context(tc.tile_pool(name="ps_mlp", bufs=4, space="PSUM"))
        ps_out = ctx.enter_context(tc.tile_pool(name="ps_out", bufs=1, space="PSUM"))

        # ---- static tensors ---------------------------------------------------
        st = {}
        # load order matters: small rp tensors + weights first, then the first
        # two supers' streaming tiles, and only THEN the 8MB rp0 (split into 8
        # independent loads so low buckets unblock early) - this way sampling
        # can begin a few us in rather than waiting behind rp0.
        order = ["rp2", "rp1", "w1", "w2", "w3h", "wout",
                 "b0", "b1", "b2", "b3", "bout"]
        specs = {n: (s, d) for n, s, d in _DRAM_SPECS}
        for name in order:
            shape, npdt = specs[name]
            t = const.tile(list(shape), mybir.dt.from_np(np.dtype(npdt)), tag=name)
            if len(shape) == 3:
                nc.sync.dma_start(t[:, :, :], dram[name][:, :, :])
            else:
                nc.sync.dma_start(t[:, :], dram[name][:, :])
            st[name] = t

        stream_tiles = {}

        def issue_stream(s):
            lo = s * SUP
            s_tiles = []
            for nm in ("s0t", "s1t", "s2t"):
                t = spool.tile([128, SUP], bf16, tag=nm, name=f"{nm}_{s}")
                nc.sync.dma_start(t[:, :], dram[nm][:, lo:lo + SUP])
                s_tiles.append(t)
            bias_t = {}
            for nm in ("bias0", "bias3"):
                t = bpool.tile([128, 2, SUP], bf16, tag=nm, name=f"{nm}_{s}")
                # dram [128, 2*N]: columns m*N + n
                nc.sync.dma_start(t[:, 0, :], dram[nm][:, lo:lo + SUP])
                nc.sync.dma_start(t[:, 1, :], dram[nm][:, N + lo:N + lo + SUP])
                bias_t[nm] = t
            stream_tiles[s] = (s_tiles, bias_t)

        shape, npdt = specs["rp0"]
        rp0_t = const.tile(list(shape), mybir.dt.from_np(np.dtype(npdt)), tag="rp0")
        st["rp0"] = rp0_t
        rp0_q = shape[1] // 8

        def issue_rp0_chunk(i):
            # supers consume rp0 buckets monotonically (samples y-sorted), so
            # chunk i is needed around super i; stream it just-in-time instead
            # of stalling early sampling behind the whole 8MB load.
            nc.sync.dma_start(rp0_t[:, i * rp0_q:(i + 1) * rp0_q],
                              dram["rp0"][:, i * rp0_q:(i + 1) * rp0_q])

        issue_stream(0)
        issue_rp0_chunk(0)
        issue_stream(1)
        issue_rp0_chunk(1)
        issue_rp0_chunk(2)

        rp = [st["rp0"], st["rp1"], st["rp2"]]
        wmlp = {"w1": st["w1"], "w2": st["w2"], "w3h": st["w3h"]}

        for s in range(NSUP):
            lo = s * SUP
            sl = slice(lo, lo + SUP)
            if s not in stream_tiles:
                issue_stream(s)
            s_tiles, bias_t = stream_tiles.pop(s)
            if s + 1 < NSUP and s + 1 not in stream_tiles:
                issue_stream(s + 1)  # a full super of DMA prefetch lead
            if s + 3 < 8:
                issue_rp0_chunk(s + 3)

            def sample_into(p, m, ch, pass_off, last_extra):
                """Accumulate the 3 pyramid levels' bucket-runs for chunk ch
                (columns of this super) into psum tile p, channel m-tile m.
                pass_off selects the layer-0 (0) or layer-3 (256) projection.
                last_extra: if True, leave the accumulation group open (caller
                adds more matmuls); else close it on the last L0 run."""
                for li in (2, 1, 0):
                    rlist = runs[li][s * NCH + ch]
                    for ri, (g, off, ln) in enumerate(rlist):
                        first = li == 2 and ri == 0
                        last = (not last_extra) and li == 0 and ri == len(rlist) - 1
                        col = g * RPC + pass_off + m * 128
                        nc.tensor.matmul(
                            p[:, off:off + ln],
                            rp[li][:, col:col + 128],
                            s_tiles[li][:, ch * CH + off: ch * CH + off + ln],
                            start=first, stop=last,
                        )

            # ---- pass 1: sampling -> +bias0 -> gelu -> h0 ---------------------
            h0 = hpool.tile([128, 2, SUP], f8e4, tag="h0")
            for m in range(2):
                for ch in range(NCH):
                    p = ps_samp.tile([128, CH], f32, tag="ps_samp")
                    sample_into(p, m, ch, 0, last_extra=False)
                    nc.vector.tensor_add(
                        p[:, :], p[:, :], bias_t["bias0"][:, m, ch * CH:(ch + 1) * CH])
                    nc.scalar.activation(
                        h0[:, m, ch * CH:(ch + 1) * CH], p[:, :], GELU,
                        bias=st["b0"][:, m:m + 1])

            # ---- dense hidden layers (fp8 DoubleRow: k=256 in one pass) -------
            def dense(wname, bname, rhs, tag):
                h = hpool.tile([128, 2, SUP], f8e4, tag=tag)
                w = wmlp[wname]
                for m in range(2):
                    for ns in range(NCH):
                        p = ps_mlp.tile([128, CH], f32, tag="ps_mlp")
                        nc.tensor.matmul(
                            p[:, :],
                            w[:, :, m * 128:(m + 1) * 128],
                            rhs[:, :, ns * CH:(ns + 1) * CH],
                            start=True, stop=True, perf_mode=DOUBLE_ROW,
                        )
                        nc.scalar.activation(
                            h[:, m, ns * CH:(ns + 1) * CH], p[:, :], GELU,
                            bias=st[bname][:, m:m + 1])
                return h

            h1 = dense("w1", "b1", h0, "h1")
            h2 = dense("w2", "b2", h1, "h2")

            # ---- layer 3: sampling pass 2 + w3h @ h2 + bias3 -> gelu -> h3 ----
            h3 = hpool.tile([128, 2, SUP], bf16, tag="h3")
            for m in range(2):
                for ch in range(NCH):
                    p = ps_samp.tile([128, CH], f32, tag="ps_samp")
                    sample_into(p, m, ch, 256, last_extra=True)
                    nc.tensor.matmul(
                        p[:, :],
                        st["w3h"][:, :, m * 128:(m + 1) * 128],
                        h2[:, :, ch * CH:(ch + 1) * CH],
                        start=False, stop=True, perf_mode=DOUBLE_ROW,
                    )
                    nc.vector.tensor_add(
                        p[:, :], p[:, :], bias_t["bias3"][:, m, ch * CH:(ch + 1) * CH])
                    nc.scalar.activation(
                        h3[:, m, ch * CH:(ch + 1) * CH], p[:, :], GELU,
                        bias=st["b3"][:, m:m + 1])

            # ---- output layer -------------------------------------------------
            osb = opool.tile([3, SUP], f32, tag="osb")
            for ns in range(NCH):
                po = ps_out.tile([128, CH], f32, tag="ps_out")
                for kt in range(2):
                    nc.tensor.matmul(
                        po[:3, :],
                        st["wout"][:, kt * 3:(kt + 1) * 3],
                        h3[:, kt, ns * CH:(ns + 1) * CH],
                        start=(kt == 0), stop=(kt == 1),
                    )
                nc.scalar.activation(
                    osb[:, ns * CH:(ns + 1) * CH], po[:3, :], TANH,
                    bias=st["bout"][:, 0:1],
                )
            nc.sync.dma_start(out_dram[:, sl], osb[:, :])

    nc.compile()
    return nc


def kernel(feature_grid, coords, w0, b0, w1, b1, w2, b2, w3, b3, w_out, b_out,
           _run_opts=None):
    from concourse.bass_utils import run_bass_kernel_spmd

    shared, per_core, perm, runs = _host_prep(
        feature_grid, coords, w0, b0, w1, b1, w2, b2, w3, b3, w_out, b_out)

    nc = _build_nc(runs)

    in_maps = []
    for b in range(B):
        m = dict(shared)
        m.update(per_core[b])
        in_maps.append(m)

    res = run_bass_kernel_spmd(
        nc, in_maps, core_ids=list(range(B)), **(_run_opts or {})
    )

    out = np.empty((B, N, 3), np.float32)
    inv = perm  # out_sorted column j corresponds to original sample perm[j]
    for b in range(B):
        out[b, inv, :] = res.results[b]["out_t"].T
    if _run_opts is not None:
        kernel._last_result = res  # for test harness introspection
    return out
